# revision 13
# baseline (speedup 1.0000x reference)
"""Trainium2 Bass kernel for nn_MapLoss (topk_masking).

Strategy
--------
The reference loss needs, per sample and per map (region / affinity), only
three reductions:

    S_tot = sum(d^2 * mask)                 (d = clamped pred - gt)
    S_pos = sum((gt > t) * d^2 * mask)
    n_pos = #(gt > t)

because for the hard-negative top-k, k = min(3*n_pos, n_neg) and whenever
k == n_neg the "top-k sum of negatives" is just S_tot - S_pos (sum of all
negatives).  The rare general branches (3*n_pos < n_neg, or n_pos == 0)
are handled by an exact host fallback per sample (statistically never
taken for this input distribution).

v2: fp16 streaming + engine-balanced compute.  The f32 baseline was
DMA-bound at ~69us (20.97MB/core @ ~330GB/s).  Inputs are cast to fp16
on the host and packed per sample into one [128, 5*2048] line (20KB per
partition per DMA), halving HBM traffic (floor ~32us).  The per-map
element passes are split across engines so none exceeds the DMA floor:

  DVE  : d = clamped_diff(pred,gt); l = d^2*m (custom, accum -> S_tot)
  Pool : (gt > t) * l               (stt, accum -> S_pos)
  Act  : sign(gt - t - eps)         (accum -> 2*n_pos - N)

Per-core output: per-partition stats tiles, final reduction on host.
Pure data parallel, 4 samples per core, no collectives.
"""

import os
import numpy as np
from contextlib import ExitStack

from concourse import bass, bacc, mybir
from concourse import tile
from concourse import bass_utils
import concourse.dve_ops as dve_ops_mod
from concourse.dve_ops import DveOp
from concourse.dve_spec import (
    Spec,
    Src0,
    Src1,
    C0,
    C1,
    Zero,
    relu,
    sq,
    lower,
    _has_src1,
)
from concourse.dve_uop import DveOpSpec
from operator import add as _op_add

# ---------------------------------------------------------------- constants
_B, _H, _W = 32, 512, 512
_N = _H * _W            # 262144 elements / sample
_P, _F = 128, 2048      # on-chip tile: 128 partitions x 2048 free  (= _N)
_NT = 5                 # tensors packed per sample (rgt, rpred, agt, apred, m)
_NCORES = 8
_SPC = _B // _NCORES    # 4 samples per core
_T_G = 0.6              # THRESH_POSITIVE_REGION
_T_A = 0.65             # THRESH_POSITIVE_AFFINITY
_LAMBDA = 2.0
_TOPK_FALLBACK = 500

# packed tensor order within a sample line: the first three (gts + mask)
# ride the sync HWDGE queue, the two preds ride the gpsimd SWDGE queue
_I_RGT, _I_AGT, _I_M, _I_RPRED, _I_APRED = range(_NT)
# sample-maps whose threshold test runs on DVE (C-maps); the rest use
# Act Sign (A-maps).  Chosen to balance DVE vs Act busy time.
_C_SET = (1, 4, 6)

# ------------------------------------------------- custom DVE op definitions


def _register_dve_op(name, spec, subdim=False):
    """Register a custom DVE op in the process-local registry (additive;
    the documented extension point is appending to dve_ops.OPS)."""
    if name in dve_ops_mod._SUB_OPCODE_FOR_NAME:
        for op in dve_ops_mod.OPS:
            if op.name == name:
                return op
        raise RuntimeError(f"{name} in opcode map but not in OPS")
    row = max(dve_ops_mod._SUB_OPCODE_FOR_NAME.values()) + 1
    assert row < 0x20, "custom DVE opcode rows exhausted"
    shas = {}
    for ver in ("v3", "v4"):
        try:
            tmp = DveOpSpec(
                name=name, opcode=row, uops=lower(spec, ver=ver),
                rd1_en=_has_src1(spec),
            )
            shas[ver] = tmp.sha(ver)
        except Exception:
            pass
    assert "v3" in shas, f"{name}: failed to lower for TRN2"
    op = DveOp(name, spec, subdim, uops_sha=shas)
    dve_ops_mod.OPS.append(op)
    dve_ops_mod._SUB_OPCODE_FOR_NAME[name] = row
    dve_ops_mod.CUSTOM_DVE_SPECS[name] = spec
    return op


_OPS_CACHE = {}


def _get_custom_ops():
    if _OPS_CACHE:
        return _OPS_CACHE

    # d = (pred - gt) - (gt > t) * relu(pred - 1)
    clamped_diff = Spec(
        body=(Src0 - Src1) - (Src1 > C0) * relu(Src0 - C1),
        reference=lambda in0, in1, s0, s1, imm2: (
            (in0.astype(np.float32) - in1.astype(np.float32))
            - (in1.astype(np.float32) > s0)
            * np.maximum(in0.astype(np.float32) - s1, 0.0)
        ).astype(np.float32),
    )

    # l = in0^2 * in1 ; accum_out = sum(l)
    def _masked_sq_ref(in0, in1, s0, s1, imm2):
        b = (np.square(in0.astype(np.float32))
             * in1.astype(np.float32)).astype(np.float32)
        return b, b.reshape(b.shape[0], -1).sum(axis=-1, keepdims=True)

    masked_sq = Spec(
        body=sq(Src0) * Src1,
        accum=_op_add,
        accum_init=Zero,
        reference=_masked_sq_ref,
    )

    _OPS_CACHE["clamped_diff"] = _register_dve_op("ANT_MAPLOSS_CLAMPED_DIFF", clamped_diff)
    _OPS_CACHE["masked_sq"] = _register_dve_op("ANT_MAPLOSS_MASKED_SQ", masked_sq)
    return _OPS_CACHE


# ------------------------------------------------------------- bass builder

_NC_CACHE = {}


def _f32_exact(x):
    return float(np.float32(x))


def _build_bass(repeats=1, loop=False, j3="pe", dtype="f16",
                bufs_in=5, bufs_work=2, bufs_ps=6):
    """j3: structure of the S_pos / S_tot reductions.
      'pe'    -- p=(gt>t) via DVE tensor_scalar (4x); d' = d*sqrt(m) via
                 TT (2x); Act Square(d') accum -> S_tot; PE trace-matmul
                 sum(p * d'^2) -> S_pos.  (mask slot carries sqrt(m))
      'ttact' -- DVE custom masked_sq + TT(l*sign) + Act copy-accum.
      'dve'   -- all three passes on DVE (baseline structure).
    ('pool' is dead: walrus rejects DVE-class instructions on Pool.)"""
    key = ("nc2", repeats, loop, j3, dtype, bufs_in, bufs_work, bufs_ps)
    if key in _NC_CACHE:
        return _NC_CACHE[key]
    ops = _get_custom_ops()

    f32 = mybir.dt.float32
    dt_in = mybir.dt.float16 if dtype == "f16" else mybir.dt.float32
    A = mybir.AluOpType

    nc = bacc.Bacc(
        "TRN2", target_bir_lowering=False, debug=False, num_devices=_NCORES
    )
    pk = nc.dram_tensor(
        "pk", [_SPC, _P, _NT, _F], dt_in, kind="ExternalInput"
    ).ap()
    # per-engine stats tiles (separate DRAM outs avoid cross-engine deps):
    #   sv[:, 2*s+mi] = S_tot          (Act square accum / DVE custom accum)
    #   sp[:, 2*s+mi] = S_pos-carrier  (DVE accum; unused in 'pe')
    #   sa[:, 2*s+mi] = n_pos-carrier  (DVE ts accum in 'pe': n_pos;
    #                                   Act sign accum else: 2*n_pos - N)
    stats_v = nc.dram_tensor("stats_v", [_P, 8], f32, kind="ExternalOutput").ap()
    stats_p = nc.dram_tensor("stats_p", [_P, 8], f32, kind="ExternalOutput").ap()
    stats_a = nc.dram_tensor("stats_a", [_P, 8], f32, kind="ExternalOutput").ap()
    if j3 == "pe":
        # W matrices, up to two [128,128] fp16 per sample-map (slots
        # 2*col, 2*col+1); host reads the diagonals
        stats_w = nc.dram_tensor(
            "stats_w", [_P, 16, 128], dt_in, kind="ExternalOutput").ap()

    with tile.TileContext(nc) as tc, ExitStack() as ctx:
        inpool = ctx.enter_context(tc.tile_pool(name="in", bufs=bufs_in))
        workpool = ctx.enter_context(tc.tile_pool(name="work", bufs=bufs_work))
        statpool = ctx.enter_context(tc.tile_pool(name="stat", bufs=1))
        pspool = None
        if j3 == "pe":
            pspool = ctx.enter_context(tc.psum_pool(name="ps", bufs=bufs_ps))

        sv = statpool.tile([_P, 8], f32)
        sp = statpool.tile([_P, 8], f32)
        sa = statpool.tile([_P, 8], f32)
        wout = None
        if j3 == "pe":
            wout = statpool.tile([_P, 16, 128], dt_in)
            nc.scalar.memzero(sv[:])
            nc.scalar.memzero(sa[:])
            nc.vector.memzero(sp[:])
        else:
            nc.scalar.memzero(sa[:])
            nc.vector.memzero(sp[:])
            nc.vector.memzero(sv[:])

        sign_bias = {}
        # eps = 2^-24 shifts the threshold off the fp16/f32 input grid
        # so sign() reproduces the strict '>' exactly (never hits 0).
        for thr in (_T_G, _T_A):
            bt = statpool.tile([_P, 1], f32, tag=f"bias{int(thr * 100)}")
            nc.gpsimd.memset(bt[:], -(_f32_exact(thr) + 2.0 ** -24))
            sign_bias[thr] = bt
        thr_full = {}
        if j3 == "pe":
            for thr in (_T_G, _T_A):
                tf = statpool.tile([_P, _F], dt_in, tag=f"thrF{int(thr * 100)}")
                nc.gpsimd.memset(tf[:], _f32_exact(thr))
                thr_full[thr] = tf

        # Engine balance (measured ns/op): DVE custom 2389, TT 1049;
        # Act ~2400-2630/op; PE trace job ~1261.  A-maps put the threshold
        # test on Act (Sign), C-maps on DVE (TT is_gt); x=3 C-maps makes
        # DVE ~= Act ~= 32us, under the ~34us DMA roof.
        # PSUM -> SBUF copies are deferred so the in-order DVE stream never
        # blocks on a PE accumulation chain: flush oldest only when >3
        # pending (PE is then several maps ahead), rest at body end.
        pending_w = []

        def flush_w(limit):
            while len(pending_w) > limit:
                ps, wc = pending_w.pop(0)
                nc.vector.tensor_copy(out=wout[:, wc], in_=ps[:])

        def pe_trace_job(lhs, rhs, wcol):
            ps = pspool.tile([_P, 128], f32, tag="w")
            for k in range(16):
                c = slice(k * 128, (k + 1) * 128)
                nc.tensor.matmul(ps[:], lhsT=lhs[:, c], rhs=rhs[:, c],
                                 start=(k == 0), stop=(k == 15))
            pending_w.append((ps, wcol))
            flush_w(3)

        def emit_sample_pe(tA, tB, s):
            maps = []
            for mi, (gi, pi, thr) in enumerate(
                ((0, 0, _T_G), (1, 1, _T_A))
            ):
                col = s * 2 + mi
                maps.append((mi, tA[:, gi], tB[:, pi], _f32_exact(thr), thr,
                             col))
            rm_t = tA[:, 2]            # holds sqrt(mask)

            # Act first: A-map sign tensors depend only on the DMA
            zs_t = {}
            for mi, gt_t, pr_t, thr32, thr, col in maps:
                if col not in _C_SET:
                    zs = workpool.tile([_P, _F], dt_in, tag=f"zs{mi}")
                    nc.scalar.activation(
                        zs[:], gt_t, mybir.ActivationFunctionType.Sign,
                        bias=sign_bias[thr][:], scale=1.0,
                        accum_out=sa[:, col : col + 1],
                    )
                    zs_t[col] = zs
            # DVE: d = (pred - gt) - (gt > t) * relu(pred - 1); d' = d*sqrt(m)
            dp_t = {}
            for mi, gt_t, pr_t, thr32, thr, col in maps:
                d = workpool.tile([_P, _F], dt_in, tag=f"d{mi}")
                nc.vector._custom_dve(
                    ops["clamped_diff"], out=d[:], in0=pr_t,
                    in1=gt_t, s0=thr32, s1=1.0,
                )
                dp = workpool.tile([_P, _F], dt_in, tag=f"dp{mi}")
                nc.vector.tensor_tensor(out=dp[:], in0=d[:], in1=rm_t,
                                        op=A.mult)
                dp_t[col] = dp
            # DVE 2x: C-map p = (gt > t) in {0,1}
            p_t = {}
            for mi, gt_t, pr_t, thr32, thr, col in maps:
                if col in _C_SET:
                    p = workpool.tile([_P, _F], dt_in, tag=f"p{mi}")
                    nc.vector.tensor_tensor(
                        out=p[:], in0=gt_t, in1=thr_full[thr][:], op=A.is_gt)
                    p_t[col] = p
            # Act: sq = d'^2 = d^2 * m, accum -> S_tot
            for mi, gt_t, pr_t, thr32, thr, col in maps:
                sq = workpool.tile([_P, _F], dt_in, tag=f"sq{mi}")
                nc.scalar.activation(
                    sq[:], dp_t[col][:],
                    mybir.ActivationFunctionType.Square,
                    bias=0.0, scale=1.0, accum_out=sv[:, col : col + 1],
                )
                # PE trace jobs
                if col in _C_SET:
                    pe_trace_job(p_t[col], sq, 2 * col)       # S_pos
                    pe_trace_job(p_t[col], p_t[col], 2 * col + 1)  # n_pos
                else:
                    pe_trace_job(zs_t[col], sq, 2 * col)      # 2*S_pos-S_tot

        def emit_map_v2(t, s, mi, gi, pi, thr):
            gt_t = t[:, gi]
            pr_t = t[:, pi]
            m_t = t[:, _I_M]
            thr32 = _f32_exact(thr)
            col = s * 2 + mi

            # Act: zs = sign(gt - t - eps), accum -> 2*n_pos - N
            zs = workpool.tile([_P, _F], dt_in, tag="zs")
            nc.scalar.activation(
                zs[:], gt_t, mybir.ActivationFunctionType.Sign,
                bias=sign_bias[thr][:], scale=1.0,
                accum_out=sa[:, col : col + 1],
            )
            # DVE: d = (pred - gt) - (gt > t) * relu(pred - 1)
            d = workpool.tile([_P, _F], dt_in, tag="d")
            nc.vector._custom_dve(
                ops["clamped_diff"], out=d[:], in0=pr_t,
                in1=gt_t, s0=thr32, s1=1.0,
            )
            # DVE: l = d^2 * m, accum -> S_tot
            l = workpool.tile([_P, _F], dt_in, tag="l")
            nc.vector._custom_dve(
                ops["masked_sq"], out=l[:], in0=d[:], in1=m_t,
                accum_out=sv[:, col : col + 1],
            )
            if j3 == "ttact":
                # u = l * zs (2x TT); Act copy-accum -> 2*S_pos - S_tot
                u = workpool.tile([_P, _F], dt_in, tag="u")
                nc.vector.tensor_tensor(
                    out=u[:], in0=l[:], in1=zs[:], op=A.mult,
                )
                dump = workpool.tile([_P, _F], dt_in, tag="dump")
                nc.scalar.activation(
                    dump[:], u[:], mybir.ActivationFunctionType.Copy,
                    bias=0.0, scale=1.0,
                    accum_out=sp[:, col : col + 1],
                )
            else:  # 'dve'
                z = workpool.tile([_P, _F], dt_in, tag="z")
                nc.vector.scalar_tensor_tensor(
                    out=z[:], in0=gt_t, scalar=thr32, in1=l[:],
                    op0=A.is_gt, op1=A.mult,
                    accum_out=sp[:, col : col + 1],
                )

        def emit_sample(s):
            if j3 == "pe":
                tA = inpool.tile([_P, 3, _F], dt_in, tag="pkA")
                nc.sync.dma_start(out=tA[:], in_=pk[s, :, 0:3])
                tB = inpool.tile([_P, 2, _F], dt_in, tag="pkB")
                nc.gpsimd.dma_start(out=tB[:], in_=pk[s, :, 3:5])
                emit_sample_pe(tA, tB, s)
                return
            t = inpool.tile([_P, _NT, _F], dt_in, tag="pk")
            nc.sync.dma_start(out=t[:], in_=pk[s])
            if False:
                pass
            else:
                for mi, (gi, pi, thr) in enumerate(
                    ((_I_RGT, _I_RPRED, _T_G), (_I_AGT, _I_APRED, _T_A))
                ):
                    emit_map_v2(t, s, mi, gi, pi, thr)

        def emit_body():
            for s in range(_SPC):
                emit_sample(s)
            if j3 == "pe":
                flush_w(0)

        if loop and repeats > 1:
            with tc.For_i(0, repeats, 1):
                emit_body()
        else:
            for _ in range(repeats):
                emit_body()

        nc.sync.dma_start(out=stats_v[:], in_=sv[:])
        nc.sync.dma_start(out=stats_p[:], in_=sp[:])
        nc.sync.dma_start(out=stats_a[:], in_=sa[:])
        if j3 == "pe":
            nc.sync.dma_start(out=stats_w[:], in_=wout[:])

    nc.compile()
    _NC_CACHE[key] = nc
    return nc


# --------------------------------------------------------- host-side packing


def _pack_inputs(arr32, j3="pe"):
    """arr32: dict of [B, P, F] float32 -> list of per-core packed
    [SPC, P, NT, F] arrays in the kernel input dtype.  For the 'pe'
    design the mask slot carries sqrt(mask) so that the device's
    Square(d * sqrt(m)) equals d^2 * m."""
    dt = np.float16
    mk = arr32["mask"]
    if j3 == "pe":
        mk = np.sqrt(mk)
    packed = []
    for c in range(_NCORES):
        sl = slice(c * _SPC, (c + 1) * _SPC)
        buf = np.empty((_SPC, _P, _NT, _F), dtype=dt)
        buf[:, :, _I_RGT] = arr32["region_score_gt"][sl]
        buf[:, :, _I_RPRED] = arr32["region_score_pred"][sl]
        buf[:, :, _I_AGT] = arr32["affinity_score_gt"][sl]
        buf[:, :, _I_APRED] = arr32["affinity_score_pred"][sl]
        buf[:, :, _I_M] = mk[sl]
        packed.append(buf)
    return packed


# ------------------------------------------------------------ host fallback


def _host_sample_loss(pre_loss, label, thresh):
    """Exact per-sample replica of reference._single_image_loss (one sample)."""
    pre_loss = pre_loss.astype(np.float64).ravel()
    label = label.astype(np.float32).ravel()
    pos_mask = label > np.float32(thresh)
    n_pos = int(pos_mask.sum())
    n_neg = pre_loss.size - n_pos
    if n_pos == 0:
        top = np.sort(pre_loss)[::-1][:_TOPK_FALLBACK]
        return float(top.mean())
    pos_loss = pre_loss[pos_mask].sum() / n_pos
    k = min(3 * n_pos, n_neg)
    if k <= 0:
        return float(pos_loss)
    neg_vals = np.sort(pre_loss[~pos_mask])[::-1]
    neg_loss = neg_vals[:k].sum() / k
    return float(pos_loss + neg_loss)


def _host_pre_loss(gt, pred, mask, thresh):
    gt = gt.astype(np.float32)
    pred = pred.astype(np.float32)
    clamped = np.where((gt > np.float32(thresh)) & (pred > np.float32(1.0)),
                       np.float32(1.0), pred)
    d = clamped.astype(np.float64) - gt.astype(np.float64)
    return d * d * mask.astype(np.float64)


# ------------------------------------------------------------------- bench


def _io_spec(nc):
    """Mirror run_bass_via_pjrt's input/output discovery."""
    partition_name = (
        nc.partition_id_tensor.name if nc.partition_id_tensor else None
    )
    in_names, out_names, out_avals, zero_outs = [], [], [], []
    import jax

    for alloc in nc.m.functions[0].allocations:
        if not isinstance(alloc, mybir.MemoryLocationSet):
            continue
        name = alloc.memorylocations[0].name
        if alloc.kind == "ExternalInput":
            if name != partition_name:
                in_names.append(name)
        elif alloc.kind == "ExternalOutput":
            out_names.append(name)
            shape = tuple(alloc.tensor_shape)
            dtype = mybir.dt.np(alloc.dtype)
            out_avals.append(jax.core.ShapedArray(shape, dtype))
            zero_outs.append(np.zeros(shape, dtype))
    return partition_name, in_names, out_names, out_avals, zero_outs


def _bench_one(inputs, iters=30, warmup=2, **build_kw):
    """Amortized per-execution wall time (ns) over `iters` queued runs."""
    import time
    import jax
    from jax.sharding import Mesh, PartitionSpec
    from jax.experimental.shard_map import shard_map
    from concourse import bass2jax
    from concourse.bass2jax import _bass_exec_p, install_neuronx_cc_hook

    install_neuronx_cc_hook()
    nc = _build_bass(**build_kw)
    pname, in_names, out_names, out_avals, zero_outs = _io_spec(nc)
    n_params, n_outs = len(in_names), len(out_names)
    all_names = in_names + out_names + ([pname] if pname else [])

    def _body(*args):
        operands = list(args)
        if pname is not None:
            operands.append(bass2jax.partition_id_tensor())
        outs = _bass_exec_p.bind(
            *operands,
            out_avals=tuple(out_avals),
            in_names=tuple(all_names),
            out_names=tuple(out_names),
            lowering_input_output_aliases=(),
            sim_require_finite=True,
            sim_require_nnan=True,
            nc=nc,
        )
        return tuple(outs)

    devices = jax.devices()[:_NCORES]
    mesh = Mesh(np.asarray(devices), ("core",))
    in_specs = (PartitionSpec("core"),) * (n_params + n_outs)
    out_specs = (PartitionSpec("core"),) * n_outs
    donate = tuple(range(n_params, n_params + n_outs))
    sharded = jax.jit(
        shard_map(_body, mesh=mesh, in_specs=in_specs, out_specs=out_specs,
                  check_rep=False),
        donate_argnums=donate, keep_unused=True,
    )

    arr32 = {k: np.ascontiguousarray(
        np.asarray(v, np.float32).reshape(_B, _P, _F))
        for k, v in inputs.items()}
    packed = _pack_inputs(arr32, j3=build_kw.get("j3", "pe"))
    assert in_names == ["pk"], in_names
    concat_in = [np.concatenate(packed, axis=0)]  # [NCORES*SPC, P, NT, F]
    dev_in = [jax.device_put(a) for a in concat_in]

    def zeros():
        return [np.zeros((_NCORES * z.shape[0], *z.shape[1:]), z.dtype)
                for z in zero_outs]

    for _ in range(warmup):
        outs = sharded(*dev_in, *zeros())
        jax.block_until_ready(outs)
    zs = [zeros() for _ in range(iters)]
    t0 = time.perf_counter()
    results = [sharded(*dev_in, *z) for z in zs]
    jax.block_until_ready(results)
    t1 = time.perf_counter()
    return (t1 - t0) / iters * 1e9


def bench(inputs, rounds=3, k_lo=400, k_hi=1200, **build_kw):
    """Device time per kernel body (ns): slope between on-device For_i loops
    of k_lo and k_hi iterations.  K must be large enough that device time
    dominates the dispatch roundtrip, else async dispatch hides it."""
    est = []
    build_kw.setdefault("j3", os.environ.get("MAPLOSS_J3", "pe"))
    for _ in range(rounds):
        lo = _bench_one(inputs, iters=4, repeats=k_lo, loop=True, **build_kw)
        hi = _bench_one(inputs, iters=4, repeats=k_hi, loop=True, **build_kw)
        est.append((hi - lo) / (k_hi - k_lo))
    return float(np.median(est))


# ------------------------------------------------------------------- kernel

LAST_RESULTS = None


def kernel(**inputs):
    global LAST_RESULTS
    arr32 = {
        k: np.ascontiguousarray(
            np.asarray(v, dtype=np.float32).reshape(_B, _P, _F))
        for k, v in inputs.items()
    }
    j3 = os.environ.get("MAPLOSS_J3", "pe")
    nc = _build_bass(j3=j3)

    packed = _pack_inputs(arr32, j3=j3)
    in_maps = [{"pk": packed[c]} for c in range(_NCORES)]

    res = bass_utils.run_bass_kernel_spmd(
        nc, in_maps, core_ids=list(range(_NCORES))
    )
    LAST_RESULTS = res

    # ---- host-side finish (tiny): per-sample scalars ----------------------
    per_sample = np.zeros((2, _B), dtype=np.float64)   # [map, sample]
    fallback_samples = []
    for c in range(_NCORES):
        sv = res.results[c]["stats_v"].astype(np.float64).sum(axis=0)  # [8]
        sp = res.results[c]["stats_p"].astype(np.float64).sum(axis=0)  # [8]
        sa = res.results[c]["stats_a"].astype(np.float64).sum(axis=0)  # [8]
        if j3 == "pe":
            w = res.results[c]["stats_w"].astype(np.float64)  # [128,16,128]
            tr = np.einsum("psp->s", w)                       # [16] diagonals
        for s in range(_SPC):
            b = c * _SPC + s
            for mi in range(2):
                col = s * 2 + mi
                S_tot = sv[col]
                if j3 == "pe":
                    if col in _C_SET:
                        S_pos = tr[2 * col]
                        n_pos_f = tr[2 * col + 1]
                    else:
                        S_pos = (tr[2 * col] + S_tot) / 2.0
                        n_pos_f = (sa[col] + _N) / 2.0
                elif j3 == "ttact":
                    # sp holds sum(l * sign) = 2*S_pos - S_tot
                    S_pos = (sp[col] + S_tot) / 2.0
                    n_pos_f = (sa[col] + _N) / 2.0
                else:
                    S_pos = sp[col]
                    n_pos_f = (sa[col] + _N) / 2.0
                n_pos = int(round(n_pos_f))
                n_neg = _N - n_pos
                ok = abs(n_pos_f - n_pos) < 1e-3
                if ok and n_pos > 0 and (n_neg == 0 or 3 * n_pos >= n_neg):
                    pos_loss = S_pos / n_pos
                    neg_loss = (S_tot - S_pos) / n_neg if n_neg > 0 else 0.0
                    per_sample[mi, b] = pos_loss + neg_loss
                else:
                    fallback_samples.append((mi, b))

    if fallback_samples:
        rgt = arr32["region_score_gt"]
        agt = arr32["affinity_score_gt"]
        rpred = arr32["region_score_pred"]
        apred = arr32["affinity_score_pred"]
        m = arr32["mask"]
        for mi, b in fallback_samples:
            if mi == 0:
                pl = _host_pre_loss(rgt[b], rpred[b], m[b], _T_G)
                per_sample[mi, b] = _host_sample_loss(pl, rgt[b], _T_G)
            else:
                pl = _host_pre_loss(agt[b], apred[b], m[b], _T_A)
                per_sample[mi, b] = _host_sample_loss(pl, agt[b], _T_A)

    char_loss = per_sample[0].sum()
    affi_loss = per_sample[1].sum()
    out = _LAMBDA * char_loss / _B + affi_loss / _B
    return np.float32(out)


# revision 16
# speedup vs baseline: 1.3382x; 1.3382x over previous
"""Trainium2 Bass kernel for nn_MapLoss (topk_masking).

Strategy
--------
The reference loss needs, per sample and per map (region / affinity), only
three reductions:

    S_tot = sum(d^2 * mask)                 (d = clamped pred - gt)
    S_pos = sum((gt > t) * d^2 * mask)
    n_pos = #(gt > t)

because for the hard-negative top-k, k = min(3*n_pos, n_neg) and whenever
k == n_neg the "top-k sum of negatives" is just S_tot - S_pos (sum of all
negatives).  The rare general branches (3*n_pos < n_neg, or n_pos == 0)
are handled by an exact host fallback per sample (statistically never
taken for this input distribution).

v3: fp16 streaming + engine-balanced compute + PE trace-reductions.
The f32 baseline was DMA-bound at ~69us (20.97MB/core @ ~305GB/s).
Host casts inputs to fp16 and packs each sample into one [128, 5*2048]
line (20KB/partition/DMA, ~34us floor); the mask slot carries sqrt(m)
so Square(d*sqrt(m)) = d^2*m.  Measured engine rates ([128,2048] fp16):
DVE custom 2389ns / TT(2x) 1049ns / stt+TS 1x ~2330ns; Act ~2400-2630ns;
PE 16-chunk trace-matmul job ~1261ns.  Per map:

  DVE  : d = clamped_diff(pred,gt) custom; d' = d*sqrt(m) TT 2x
  Act  : sq = Square(d') accum -> S_tot
  A-maps (5): Act Sign(gt-t-eps)=zs accum -> 2*n_pos-N;
              PE trace sum_k zs_k^T sq_k -> 2*S_pos-S_tot
  C-maps (3): DVE TT (gt > thr_tile) = p (0/1);
              PE traces p^T sq -> S_pos and p^T p -> n_pos

The PSUM->SBUF copies of the PE trace matrices are deferred several maps
(>3 pending) so the in-order DVE stream never waits on a PE chain.
A/C split (x=3) balances DVE ~= Act ~= 32us under the ~34us DMA roof;
bufs_in=5 decouples the input-DMA ring from engine lag.
Per-core output: per-partition stats + W matrices; host sums/diagonals.
Pure data parallel, 4 samples per core, no collectives.
"""

import os
import numpy as np
from contextlib import ExitStack

from concourse import bass, bacc, mybir
from concourse import tile
from concourse import bass_utils
import concourse.dve_ops as dve_ops_mod
from concourse.dve_ops import DveOp
from concourse.dve_spec import (
    Spec,
    Src0,
    Src1,
    C0,
    C1,
    Zero,
    relu,
    sq,
    lower,
    _has_src1,
)
from concourse.dve_uop import DveOpSpec
from operator import add as _op_add

# ---------------------------------------------------------------- constants
_B, _H, _W = 32, 512, 512
_N = _H * _W            # 262144 elements / sample
_P, _F = 128, 2048      # on-chip tile: 128 partitions x 2048 free  (= _N)
_NT = 5                 # tensors packed per sample (rgt, rpred, agt, apred, m)
_NCORES = 8
_SPC = _B // _NCORES    # 4 samples per core
_T_G = 0.6              # THRESH_POSITIVE_REGION
_T_A = 0.65             # THRESH_POSITIVE_AFFINITY
_LAMBDA = 2.0
_TOPK_FALLBACK = 500

# packed tensor order within a sample line: the first three (gts + mask)
# ride the sync HWDGE queue, the two preds ride the gpsimd SWDGE queue
_I_RGT, _I_AGT, _I_M, _I_RPRED, _I_APRED = range(_NT)
# sample-maps whose threshold test runs on DVE (C-maps); the rest use
# Act Sign (A-maps).  Chosen to balance DVE vs Act busy time.
_C_SET = (1, 4, 6)

# ------------------------------------------------- custom DVE op definitions


def _register_dve_op(name, spec, subdim=False):
    """Register a custom DVE op in the process-local registry (additive;
    the documented extension point is appending to dve_ops.OPS)."""
    if name in dve_ops_mod._SUB_OPCODE_FOR_NAME:
        for op in dve_ops_mod.OPS:
            if op.name == name:
                return op
        raise RuntimeError(f"{name} in opcode map but not in OPS")
    row = max(dve_ops_mod._SUB_OPCODE_FOR_NAME.values()) + 1
    assert row < 0x20, "custom DVE opcode rows exhausted"
    shas = {}
    for ver in ("v3", "v4"):
        try:
            tmp = DveOpSpec(
                name=name, opcode=row, uops=lower(spec, ver=ver),
                rd1_en=_has_src1(spec),
            )
            shas[ver] = tmp.sha(ver)
        except Exception:
            pass
    assert "v3" in shas, f"{name}: failed to lower for TRN2"
    op = DveOp(name, spec, subdim, uops_sha=shas)
    dve_ops_mod.OPS.append(op)
    dve_ops_mod._SUB_OPCODE_FOR_NAME[name] = row
    dve_ops_mod.CUSTOM_DVE_SPECS[name] = spec
    return op


_OPS_CACHE = {}


def _get_custom_ops():
    if _OPS_CACHE:
        return _OPS_CACHE

    # d = (pred - gt) - (gt > t) * relu(pred - 1)
    clamped_diff = Spec(
        body=(Src0 - Src1) - (Src1 > C0) * relu(Src0 - C1),
        reference=lambda in0, in1, s0, s1, imm2: (
            (in0.astype(np.float32) - in1.astype(np.float32))
            - (in1.astype(np.float32) > s0)
            * np.maximum(in0.astype(np.float32) - s1, 0.0)
        ).astype(np.float32),
    )

    # l = in0^2 * in1 ; accum_out = sum(l)
    def _masked_sq_ref(in0, in1, s0, s1, imm2):
        b = (np.square(in0.astype(np.float32))
             * in1.astype(np.float32)).astype(np.float32)
        return b, b.reshape(b.shape[0], -1).sum(axis=-1, keepdims=True)

    masked_sq = Spec(
        body=sq(Src0) * Src1,
        accum=_op_add,
        accum_init=Zero,
        reference=_masked_sq_ref,
    )

    _OPS_CACHE["clamped_diff"] = _register_dve_op("ANT_MAPLOSS_CLAMPED_DIFF", clamped_diff)
    _OPS_CACHE["masked_sq"] = _register_dve_op("ANT_MAPLOSS_MASKED_SQ", masked_sq)
    return _OPS_CACHE


# ------------------------------------------------------------- bass builder

_NC_CACHE = {}


def _f32_exact(x):
    return float(np.float32(x))


def _build_bass(repeats=1, loop=False, j3="pe", dtype="f16",
                bufs_in=5, bufs_work=2, bufs_ps=6, q2="sync",
                tiles="one"):
    """j3: structure of the S_pos / S_tot reductions.
      'pe'    -- p=(gt>t) via DVE tensor_scalar (4x); d' = d*sqrt(m) via
                 TT (2x); Act Square(d') accum -> S_tot; PE trace-matmul
                 sum(p * d'^2) -> S_pos.  (mask slot carries sqrt(m))
      'ttact' -- DVE custom masked_sq + TT(l*sign) + Act copy-accum.
      'dve'   -- all three passes on DVE (baseline structure).
    ('pool' is dead: walrus rejects DVE-class instructions on Pool.)"""
    key = ("nc2", repeats, loop, j3, dtype, bufs_in, bufs_work, bufs_ps, q2,
           tiles)
    if key in _NC_CACHE:
        return _NC_CACHE[key]
    ops = _get_custom_ops()

    f32 = mybir.dt.float32
    dt_in = mybir.dt.float16 if dtype == "f16" else mybir.dt.float32
    A = mybir.AluOpType

    nc = bacc.Bacc(
        "TRN2", target_bir_lowering=False, debug=False, num_devices=_NCORES
    )
    pk = nc.dram_tensor(
        "pk", [_SPC, _P, _NT, _F], dt_in, kind="ExternalInput"
    ).ap()
    # per-engine stats tiles (separate DRAM outs avoid cross-engine deps):
    #   sv[:, 2*s+mi] = S_tot          (Act square accum / DVE custom accum)
    #   sp[:, 2*s+mi] = S_pos-carrier  (DVE accum; unused in 'pe')
    #   sa[:, 2*s+mi] = n_pos-carrier  (DVE ts accum in 'pe': n_pos;
    #                                   Act sign accum else: 2*n_pos - N)
    stats_v = nc.dram_tensor("stats_v", [_P, 8], f32, kind="ExternalOutput").ap()
    stats_p = nc.dram_tensor("stats_p", [_P, 8], f32, kind="ExternalOutput").ap()
    stats_a = nc.dram_tensor("stats_a", [_P, 8], f32, kind="ExternalOutput").ap()
    if j3 == "pe":
        # W matrices, up to two [128,128] fp16 per sample-map (slots
        # 2*col, 2*col+1); host reads the diagonals
        stats_w = nc.dram_tensor(
            "stats_w", [_P, 16, 128], dt_in, kind="ExternalOutput").ap()

    with tile.TileContext(nc) as tc, ExitStack() as ctx:
        inpool = ctx.enter_context(tc.tile_pool(name="in", bufs=bufs_in))
        workpool = ctx.enter_context(tc.tile_pool(name="work", bufs=bufs_work))
        statpool = ctx.enter_context(tc.tile_pool(name="stat", bufs=1))
        pspool = None
        if j3 == "pe":
            pspool = ctx.enter_context(tc.psum_pool(name="ps", bufs=bufs_ps))

        sv = statpool.tile([_P, 8], f32)
        sp = statpool.tile([_P, 8], f32)
        sa = statpool.tile([_P, 8], f32)
        wout = None
        if j3 == "pe":
            wout = statpool.tile([_P, 16, 128], dt_in)
            nc.scalar.memzero(sv[:])
            nc.scalar.memzero(sa[:])
            nc.vector.memzero(sp[:])
        else:
            nc.scalar.memzero(sa[:])
            nc.vector.memzero(sp[:])
            nc.vector.memzero(sv[:])

        sign_bias = {}
        # eps = 2^-24 shifts the threshold off the fp16/f32 input grid
        # so sign() reproduces the strict '>' exactly (never hits 0).
        for thr in (_T_G, _T_A):
            bt = statpool.tile([_P, 1], f32, tag=f"bias{int(thr * 100)}")
            nc.gpsimd.memset(bt[:], -(_f32_exact(thr) + 2.0 ** -24))
            sign_bias[thr] = bt
        thr_full = {}
        if j3 == "pe":
            for thr in (_T_G, _T_A):
                tf = statpool.tile([_P, _F], dt_in, tag=f"thrF{int(thr * 100)}")
                nc.gpsimd.memset(tf[:], _f32_exact(thr))
                thr_full[thr] = tf

        # Engine balance (measured ns/op): DVE custom 2389, TT 1049;
        # Act ~2400-2630/op; PE trace job ~1261.  A-maps put the threshold
        # test on Act (Sign), C-maps on DVE (TT is_gt); x=3 C-maps makes
        # DVE ~= Act ~= 32us, under the ~34us DMA roof.
        # PSUM -> SBUF copies are deferred so the in-order DVE stream never
        # blocks on a PE accumulation chain: flush oldest only when >3
        # pending (PE is then several maps ahead), rest at body end.
        pending_w = []

        def flush_w(limit):
            while len(pending_w) > limit:
                ps, wc = pending_w.pop(0)
                nc.vector.tensor_copy(out=wout[:, wc], in_=ps[:])

        def pe_trace_job(lhs, rhs, wcol):
            ps = pspool.tile([_P, 128], f32, tag="w")
            for k in range(16):
                c = slice(k * 128, (k + 1) * 128)
                nc.tensor.matmul(ps[:], lhsT=lhs[:, c], rhs=rhs[:, c],
                                 start=(k == 0), stop=(k == 15))
            pending_w.append((ps, wcol))
            flush_w(3)

        def emit_sample_pe5(sub, s):
            maps = [(0, sub[0][:], sub[3][:], _f32_exact(_T_G), _T_G, s * 2),
                    (1, sub[1][:], sub[4][:], _f32_exact(_T_A), _T_A,
                     s * 2 + 1)]
            _emit_maps_pe(maps, sub[2][:], s)

        def emit_sample_pe(tA, tB, rm_ap, s):
            maps = []
            for mi, (gi, pi, thr) in enumerate(
                ((0, 0, _T_G), (1, 1, _T_A))
            ):
                col = s * 2 + mi
                maps.append((mi, tA[:, gi], tB[:, pi], _f32_exact(thr), thr,
                             col))
            _emit_maps_pe(maps, rm_ap, s)

        def _emit_maps_pe(maps, rm_t, s):
            # Act first: A-map sign tensors depend only on the DMA
            zs_t = {}
            for mi, gt_t, pr_t, thr32, thr, col in maps:
                if col not in _C_SET:
                    zs = workpool.tile([_P, _F], dt_in, tag=f"zs{mi}")
                    nc.scalar.activation(
                        zs[:], gt_t, mybir.ActivationFunctionType.Sign,
                        bias=sign_bias[thr][:], scale=1.0,
                        accum_out=sa[:, col : col + 1],
                    )
                    zs_t[col] = zs
            # DVE: d = (pred - gt) - (gt > t) * relu(pred - 1); d' = d*sqrt(m)
            dp_t = {}
            for mi, gt_t, pr_t, thr32, thr, col in maps:
                d = workpool.tile([_P, _F], dt_in, tag=f"d{mi}")
                nc.vector._custom_dve(
                    ops["clamped_diff"], out=d[:], in0=pr_t,
                    in1=gt_t, s0=thr32, s1=1.0,
                )
                dp = workpool.tile([_P, _F], dt_in, tag=f"dp{mi}")
                nc.vector.tensor_tensor(out=dp[:], in0=d[:], in1=rm_t,
                                        op=A.mult)
                dp_t[col] = dp
            # DVE 2x: C-map p = (gt > t) in {0,1}
            p_t = {}
            for mi, gt_t, pr_t, thr32, thr, col in maps:
                if col in _C_SET:
                    p = workpool.tile([_P, _F], dt_in, tag=f"p{mi}")
                    nc.vector.tensor_tensor(
                        out=p[:], in0=gt_t, in1=thr_full[thr][:], op=A.is_gt)
                    p_t[col] = p
            # Act: sq = d'^2 = d^2 * m, accum -> S_tot
            for mi, gt_t, pr_t, thr32, thr, col in maps:
                sq = workpool.tile([_P, _F], dt_in, tag=f"sq{mi}")
                nc.scalar.activation(
                    sq[:], dp_t[col][:],
                    mybir.ActivationFunctionType.Square,
                    bias=0.0, scale=1.0, accum_out=sv[:, col : col + 1],
                )
                # PE trace jobs
                if col in _C_SET:
                    pe_trace_job(p_t[col], sq, 2 * col)       # S_pos
                    pe_trace_job(p_t[col], p_t[col], 2 * col + 1)  # n_pos
                else:
                    pe_trace_job(zs_t[col], sq, 2 * col)      # 2*S_pos-S_tot

        def emit_map_v2(t, s, mi, gi, pi, thr):
            gt_t = t[:, gi]
            pr_t = t[:, pi]
            m_t = t[:, _I_M]
            thr32 = _f32_exact(thr)
            col = s * 2 + mi

            # Act: zs = sign(gt - t - eps), accum -> 2*n_pos - N
            zs = workpool.tile([_P, _F], dt_in, tag="zs")
            nc.scalar.activation(
                zs[:], gt_t, mybir.ActivationFunctionType.Sign,
                bias=sign_bias[thr][:], scale=1.0,
                accum_out=sa[:, col : col + 1],
            )
            # DVE: d = (pred - gt) - (gt > t) * relu(pred - 1)
            d = workpool.tile([_P, _F], dt_in, tag="d")
            nc.vector._custom_dve(
                ops["clamped_diff"], out=d[:], in0=pr_t,
                in1=gt_t, s0=thr32, s1=1.0,
            )
            # DVE: l = d^2 * m, accum -> S_tot
            l = workpool.tile([_P, _F], dt_in, tag="l")
            nc.vector._custom_dve(
                ops["masked_sq"], out=l[:], in0=d[:], in1=m_t,
                accum_out=sv[:, col : col + 1],
            )
            if j3 == "ttact":
                # u = l * zs (2x TT); Act copy-accum -> 2*S_pos - S_tot
                u = workpool.tile([_P, _F], dt_in, tag="u")
                nc.vector.tensor_tensor(
                    out=u[:], in0=l[:], in1=zs[:], op=A.mult,
                )
                dump = workpool.tile([_P, _F], dt_in, tag="dump")
                nc.scalar.activation(
                    dump[:], u[:], mybir.ActivationFunctionType.Copy,
                    bias=0.0, scale=1.0,
                    accum_out=sp[:, col : col + 1],
                )
            else:  # 'dve'
                z = workpool.tile([_P, _F], dt_in, tag="z")
                nc.vector.scalar_tensor_tensor(
                    out=z[:], in0=gt_t, scalar=thr32, in1=l[:],
                    op0=A.is_gt, op1=A.mult,
                    accum_out=sp[:, col : col + 1],
                )

        def emit_sample(s):
            if j3 == "pe":
                q2_eng = {"sync": nc.sync, "scalar": nc.scalar,
                          "gpsimd": nc.gpsimd}[q2]
                if tiles == "one":
                    tt = inpool.tile([_P, _NT, _F], dt_in, tag="pk1")
                    nc.sync.dma_start(out=tt[:], in_=pk[s])
                    tA, tB, rm_ap = tt[:, 0:3], tt[:, 3:5], tt[:, 2]
                elif tiles == "five":
                    sub = []
                    for j, tg in enumerate(("tgr", "tga", "trm", "tpr", "tpa")):
                        st = inpool.tile([_P, _F], dt_in, tag=tg)
                        eng = q2_eng if j >= 3 else nc.sync
                        eng.dma_start(out=st[:], in_=pk[s, :, j])
                        sub.append(st)
                    tA, tB, rm_ap = None, None, None
                    emit_sample_pe5(sub, s)
                    return
                else:  # "two"
                    tA_t = inpool.tile([_P, 3, _F], dt_in, tag="pkA")
                    nc.sync.dma_start(out=tA_t[:], in_=pk[s, :, 0:3])
                    tB_t = inpool.tile([_P, 2, _F], dt_in, tag="pkB")
                    q2_eng.dma_start(out=tB_t[:], in_=pk[s, :, 3:5])
                    tA, tB, rm_ap = tA_t[:], tB_t[:], tA_t[:, 2]
                emit_sample_pe(tA, tB, rm_ap, s)
                return
            t = inpool.tile([_P, _NT, _F], dt_in, tag="pk")
            nc.sync.dma_start(out=t[:], in_=pk[s])
            if False:
                pass
            else:
                for mi, (gi, pi, thr) in enumerate(
                    ((_I_RGT, _I_RPRED, _T_G), (_I_AGT, _I_APRED, _T_A))
                ):
                    emit_map_v2(t, s, mi, gi, pi, thr)

        def emit_body():
            for s in range(_SPC):
                emit_sample(s)
            if j3 == "pe":
                flush_w(0)

        if loop and repeats > 1:
            with tc.For_i(0, repeats, 1):
                emit_body()
        else:
            for _ in range(repeats):
                emit_body()

        nc.sync.dma_start(out=stats_v[:], in_=sv[:])
        nc.sync.dma_start(out=stats_p[:], in_=sp[:])
        nc.sync.dma_start(out=stats_a[:], in_=sa[:])
        if j3 == "pe":
            nc.sync.dma_start(out=stats_w[:], in_=wout[:])

    nc.compile()
    _NC_CACHE[key] = nc
    return nc


# --------------------------------------------------------- host-side packing


def _pack_inputs(arr32, j3="pe"):
    """arr32: dict of [B, P, F] float32 -> list of per-core packed
    [SPC, P, NT, F] arrays in the kernel input dtype.  For the 'pe'
    design the mask slot carries sqrt(mask) so that the device's
    Square(d * sqrt(m)) equals d^2 * m."""
    dt = np.float16
    mk = arr32["mask"]
    if j3 == "pe":
        mk = np.sqrt(mk)
    packed = []
    for c in range(_NCORES):
        sl = slice(c * _SPC, (c + 1) * _SPC)
        buf = np.empty((_SPC, _P, _NT, _F), dtype=dt)
        buf[:, :, _I_RGT] = arr32["region_score_gt"][sl]
        buf[:, :, _I_RPRED] = arr32["region_score_pred"][sl]
        buf[:, :, _I_AGT] = arr32["affinity_score_gt"][sl]
        buf[:, :, _I_APRED] = arr32["affinity_score_pred"][sl]
        buf[:, :, _I_M] = mk[sl]
        packed.append(buf)
    return packed


# ------------------------------------------------------------ host fallback


def _host_sample_loss(pre_loss, label, thresh):
    """Exact per-sample replica of reference._single_image_loss (one sample)."""
    pre_loss = pre_loss.astype(np.float64).ravel()
    label = label.astype(np.float32).ravel()
    pos_mask = label > np.float32(thresh)
    n_pos = int(pos_mask.sum())
    n_neg = pre_loss.size - n_pos
    if n_pos == 0:
        top = np.sort(pre_loss)[::-1][:_TOPK_FALLBACK]
        return float(top.mean())
    pos_loss = pre_loss[pos_mask].sum() / n_pos
    k = min(3 * n_pos, n_neg)
    if k <= 0:
        return float(pos_loss)
    neg_vals = np.sort(pre_loss[~pos_mask])[::-1]
    neg_loss = neg_vals[:k].sum() / k
    return float(pos_loss + neg_loss)


def _host_pre_loss(gt, pred, mask, thresh):
    gt = gt.astype(np.float32)
    pred = pred.astype(np.float32)
    clamped = np.where((gt > np.float32(thresh)) & (pred > np.float32(1.0)),
                       np.float32(1.0), pred)
    d = clamped.astype(np.float64) - gt.astype(np.float64)
    return d * d * mask.astype(np.float64)


# ------------------------------------------------------------------- bench


def _io_spec(nc):
    """Mirror run_bass_via_pjrt's input/output discovery."""
    partition_name = (
        nc.partition_id_tensor.name if nc.partition_id_tensor else None
    )
    in_names, out_names, out_avals, zero_outs = [], [], [], []
    import jax

    for alloc in nc.m.functions[0].allocations:
        if not isinstance(alloc, mybir.MemoryLocationSet):
            continue
        name = alloc.memorylocations[0].name
        if alloc.kind == "ExternalInput":
            if name != partition_name:
                in_names.append(name)
        elif alloc.kind == "ExternalOutput":
            out_names.append(name)
            shape = tuple(alloc.tensor_shape)
            dtype = mybir.dt.np(alloc.dtype)
            out_avals.append(jax.core.ShapedArray(shape, dtype))
            zero_outs.append(np.zeros(shape, dtype))
    return partition_name, in_names, out_names, out_avals, zero_outs


def _bench_one(inputs, iters=30, warmup=2, **build_kw):
    """Amortized per-execution wall time (ns) over `iters` queued runs."""
    import time
    import jax
    from jax.sharding import Mesh, PartitionSpec
    from jax.experimental.shard_map import shard_map
    from concourse import bass2jax
    from concourse.bass2jax import _bass_exec_p, install_neuronx_cc_hook

    install_neuronx_cc_hook()
    nc = _build_bass(**build_kw)
    pname, in_names, out_names, out_avals, zero_outs = _io_spec(nc)
    n_params, n_outs = len(in_names), len(out_names)
    all_names = in_names + out_names + ([pname] if pname else [])

    def _body(*args):
        operands = list(args)
        if pname is not None:
            operands.append(bass2jax.partition_id_tensor())
        outs = _bass_exec_p.bind(
            *operands,
            out_avals=tuple(out_avals),
            in_names=tuple(all_names),
            out_names=tuple(out_names),
            lowering_input_output_aliases=(),
            sim_require_finite=True,
            sim_require_nnan=True,
            nc=nc,
        )
        return tuple(outs)

    devices = jax.devices()[:_NCORES]
    mesh = Mesh(np.asarray(devices), ("core",))
    in_specs = (PartitionSpec("core"),) * (n_params + n_outs)
    out_specs = (PartitionSpec("core"),) * n_outs
    donate = tuple(range(n_params, n_params + n_outs))
    sharded = jax.jit(
        shard_map(_body, mesh=mesh, in_specs=in_specs, out_specs=out_specs,
                  check_rep=False),
        donate_argnums=donate, keep_unused=True,
    )

    arr32 = {k: np.ascontiguousarray(
        np.asarray(v, np.float32).reshape(_B, _P, _F))
        for k, v in inputs.items()}
    packed = _pack_inputs(arr32, j3=build_kw.get("j3", "pe"))
    assert in_names == ["pk"], in_names
    concat_in = [np.concatenate(packed, axis=0)]  # [NCORES*SPC, P, NT, F]
    dev_in = [jax.device_put(a) for a in concat_in]

    def zeros():
        return [np.zeros((_NCORES * z.shape[0], *z.shape[1:]), z.dtype)
                for z in zero_outs]

    for _ in range(warmup):
        outs = sharded(*dev_in, *zeros())
        jax.block_until_ready(outs)
    zs = [zeros() for _ in range(iters)]
    t0 = time.perf_counter()
    results = [sharded(*dev_in, *z) for z in zs]
    jax.block_until_ready(results)
    t1 = time.perf_counter()
    return (t1 - t0) / iters * 1e9


def bench(inputs, rounds=3, k_lo=400, k_hi=1200, **build_kw):
    """Device time per kernel body (ns): slope between on-device For_i loops
    of k_lo and k_hi iterations.  K must be large enough that device time
    dominates the dispatch roundtrip, else async dispatch hides it."""
    est = []
    build_kw.setdefault("j3", os.environ.get("MAPLOSS_J3", "pe"))
    for _ in range(rounds):
        lo = _bench_one(inputs, iters=4, repeats=k_lo, loop=True, **build_kw)
        hi = _bench_one(inputs, iters=4, repeats=k_hi, loop=True, **build_kw)
        est.append((hi - lo) / (k_hi - k_lo))
    return float(np.median(est))


# ------------------------------------------------------------------- kernel

LAST_RESULTS = None


def kernel(**inputs):
    global LAST_RESULTS
    arr32 = {
        k: np.ascontiguousarray(
            np.asarray(v, dtype=np.float32).reshape(_B, _P, _F))
        for k, v in inputs.items()
    }
    j3 = os.environ.get("MAPLOSS_J3", "pe")
    nc = _build_bass(j3=j3)

    packed = _pack_inputs(arr32, j3=j3)
    in_maps = [{"pk": packed[c]} for c in range(_NCORES)]

    res = bass_utils.run_bass_kernel_spmd(
        nc, in_maps, core_ids=list(range(_NCORES))
    )
    LAST_RESULTS = res

    # ---- host-side finish (tiny): per-sample scalars ----------------------
    per_sample = np.zeros((2, _B), dtype=np.float64)   # [map, sample]
    fallback_samples = []
    for c in range(_NCORES):
        sv = res.results[c]["stats_v"].astype(np.float64).sum(axis=0)  # [8]
        sp = res.results[c]["stats_p"].astype(np.float64).sum(axis=0)  # [8]
        sa = res.results[c]["stats_a"].astype(np.float64).sum(axis=0)  # [8]
        if j3 == "pe":
            w = res.results[c]["stats_w"].astype(np.float64)  # [128,16,128]
            tr = np.einsum("psp->s", w)                       # [16] diagonals
        for s in range(_SPC):
            b = c * _SPC + s
            for mi in range(2):
                col = s * 2 + mi
                S_tot = sv[col]
                if j3 == "pe":
                    if col in _C_SET:
                        S_pos = tr[2 * col]
                        n_pos_f = tr[2 * col + 1]
                    else:
                        S_pos = (tr[2 * col] + S_tot) / 2.0
                        n_pos_f = (sa[col] + _N) / 2.0
                elif j3 == "ttact":
                    # sp holds sum(l * sign) = 2*S_pos - S_tot
                    S_pos = (sp[col] + S_tot) / 2.0
                    n_pos_f = (sa[col] + _N) / 2.0
                else:
                    S_pos = sp[col]
                    n_pos_f = (sa[col] + _N) / 2.0
                n_pos = int(round(n_pos_f))
                n_neg = _N - n_pos
                ok = abs(n_pos_f - n_pos) < 1e-3
                if ok and n_pos > 0 and (n_neg == 0 or 3 * n_pos >= n_neg):
                    pos_loss = S_pos / n_pos
                    neg_loss = (S_tot - S_pos) / n_neg if n_neg > 0 else 0.0
                    per_sample[mi, b] = pos_loss + neg_loss
                else:
                    fallback_samples.append((mi, b))

    if fallback_samples:
        rgt = arr32["region_score_gt"]
        agt = arr32["affinity_score_gt"]
        rpred = arr32["region_score_pred"]
        apred = arr32["affinity_score_pred"]
        m = arr32["mask"]
        for mi, b in fallback_samples:
            if mi == 0:
                pl = _host_pre_loss(rgt[b], rpred[b], m[b], _T_G)
                per_sample[mi, b] = _host_sample_loss(pl, rgt[b], _T_G)
            else:
                pl = _host_pre_loss(agt[b], apred[b], m[b], _T_A)
                per_sample[mi, b] = _host_sample_loss(pl, agt[b], _T_A)

    char_loss = per_sample[0].sum()
    affi_loss = per_sample[1].sum()
    out = _LAMBDA * char_loss / _B + affi_loss / _B
    return np.float32(out)


# revision 18
# speedup vs baseline: 1.3418x; 1.0027x over previous
"""Trainium2 Bass kernel for nn_MapLoss (topk_masking).

Strategy
--------
The reference loss needs, per sample and per map (region / affinity), only
three reductions:

    S_tot = sum(d^2 * mask)                 (d = clamped pred - gt)
    S_pos = sum((gt > t) * d^2 * mask)
    n_pos = #(gt > t)

because for the hard-negative top-k, k = min(3*n_pos, n_neg) and whenever
k == n_neg the "top-k sum of negatives" is just S_tot - S_pos (sum of all
negatives).  The rare general branches (3*n_pos < n_neg, or n_pos == 0)
are handled by an exact host fallback per sample (statistically never
taken for this input distribution).

v3: fp16 streaming + engine-balanced compute + PE trace-reductions.
The f32 baseline was DMA-bound at ~69us (20.97MB/core @ ~305GB/s).
Host casts inputs to fp16 and packs each sample into one [128, 5*2048]
line (20KB/partition/DMA, ~34us floor); the mask slot carries sqrt(m)
so Square(d*sqrt(m)) = d^2*m.  Measured engine rates ([128,2048] fp16):
DVE custom 2389ns / TT(2x) 1049ns / stt+TS 1x ~2330ns; Act ~2400-2630ns;
PE 16-chunk trace-matmul job ~1261ns.  Per map:

  DVE  : d = clamped_diff(pred,gt) custom; d' = d*sqrt(m) TT 2x
  Act  : sq = Square(d') accum -> S_tot
  A-maps (5): Act Sign(gt-t-eps)=zs accum -> 2*n_pos-N;
              PE trace sum_k zs_k^T sq_k -> 2*S_pos-S_tot
  C-maps (3): DVE TT (gt > thr_tile) = p (0/1);
              PE traces p^T sq -> S_pos and p^T p -> n_pos

The PSUM->SBUF copies of the PE trace matrices are deferred several maps
(>3 pending) so the in-order DVE stream never waits on a PE chain.
A/C split (x=3) balances DVE ~= Act ~= 32us under the ~34us DMA roof;
bufs_in=5 decouples the input-DMA ring from engine lag.
Per-core output: per-partition stats + W matrices; host sums/diagonals.
Pure data parallel, 4 samples per core, no collectives.
"""

import os
import numpy as np
from contextlib import ExitStack

from concourse import bass, bacc, mybir
from concourse import tile
from concourse import bass_utils
import concourse.dve_ops as dve_ops_mod
from concourse.dve_ops import DveOp
from concourse.dve_spec import (
    Spec,
    Src0,
    Src1,
    C0,
    C1,
    Zero,
    relu,
    sq,
    lower,
    _has_src1,
)
from concourse.dve_uop import DveOpSpec
from operator import add as _op_add

# ---------------------------------------------------------------- constants
_B, _H, _W = 32, 512, 512
_N = _H * _W            # 262144 elements / sample
_P, _F = 128, 2048      # on-chip tile: 128 partitions x 2048 free  (= _N)
_NT = 5                 # tensors packed per sample (rgt, rpred, agt, apred, m)
_NCORES = 8
_SPC = _B // _NCORES    # 4 samples per core
_T_G = 0.6              # THRESH_POSITIVE_REGION
_T_A = 0.65             # THRESH_POSITIVE_AFFINITY
_LAMBDA = 2.0
_TOPK_FALLBACK = 500

# packed tensor order within a sample line: the first three (gts + mask)
# ride the sync HWDGE queue, the two preds ride the gpsimd SWDGE queue
_I_RGT, _I_AGT, _I_M, _I_RPRED, _I_APRED = range(_NT)
# sample-maps whose threshold test runs on DVE (C-maps); the rest use
# Act Sign (A-maps).  Chosen to balance DVE vs Act busy time.
_C_SET = (1, 4, 6)

# ------------------------------------------------- custom DVE op definitions


def _register_dve_op(name, spec, subdim=False):
    """Register a custom DVE op in the process-local registry (additive;
    the documented extension point is appending to dve_ops.OPS)."""
    if name in dve_ops_mod._SUB_OPCODE_FOR_NAME:
        for op in dve_ops_mod.OPS:
            if op.name == name:
                return op
        raise RuntimeError(f"{name} in opcode map but not in OPS")
    row = max(dve_ops_mod._SUB_OPCODE_FOR_NAME.values()) + 1
    assert row < 0x20, "custom DVE opcode rows exhausted"
    shas = {}
    for ver in ("v3", "v4"):
        try:
            tmp = DveOpSpec(
                name=name, opcode=row, uops=lower(spec, ver=ver),
                rd1_en=_has_src1(spec),
            )
            shas[ver] = tmp.sha(ver)
        except Exception:
            pass
    assert "v3" in shas, f"{name}: failed to lower for TRN2"
    op = DveOp(name, spec, subdim, uops_sha=shas)
    dve_ops_mod.OPS.append(op)
    dve_ops_mod._SUB_OPCODE_FOR_NAME[name] = row
    dve_ops_mod.CUSTOM_DVE_SPECS[name] = spec
    return op


_OPS_CACHE = {}


def _get_custom_ops():
    if _OPS_CACHE:
        return _OPS_CACHE

    # d = (pred - gt) - (gt > t) * relu(pred - 1)
    clamped_diff = Spec(
        body=(Src0 - Src1) - (Src1 > C0) * relu(Src0 - C1),
        reference=lambda in0, in1, s0, s1, imm2: (
            (in0.astype(np.float32) - in1.astype(np.float32))
            - (in1.astype(np.float32) > s0)
            * np.maximum(in0.astype(np.float32) - s1, 0.0)
        ).astype(np.float32),
    )

    # l = in0^2 * in1 ; accum_out = sum(l)
    def _masked_sq_ref(in0, in1, s0, s1, imm2):
        b = (np.square(in0.astype(np.float32))
             * in1.astype(np.float32)).astype(np.float32)
        return b, b.reshape(b.shape[0], -1).sum(axis=-1, keepdims=True)

    masked_sq = Spec(
        body=sq(Src0) * Src1,
        accum=_op_add,
        accum_init=Zero,
        reference=_masked_sq_ref,
    )

    _OPS_CACHE["clamped_diff"] = _register_dve_op("ANT_MAPLOSS_CLAMPED_DIFF", clamped_diff)
    _OPS_CACHE["masked_sq"] = _register_dve_op("ANT_MAPLOSS_MASKED_SQ", masked_sq)
    return _OPS_CACHE


# ------------------------------------------------------------- bass builder

_NC_CACHE = {}


def _f32_exact(x):
    return float(np.float32(x))


def _build_bass(repeats=1, loop=False, j3="pe", dtype="f16",
                bufs_in=5, bufs_work=2, bufs_ps=6, q2="sync",
                tiles="one"):
    """j3: structure of the S_pos / S_tot reductions.
      'pe'    -- p=(gt>t) via DVE tensor_scalar (4x); d' = d*sqrt(m) via
                 TT (2x); Act Square(d') accum -> S_tot; PE trace-matmul
                 sum(p * d'^2) -> S_pos.  (mask slot carries sqrt(m))
      'ttact' -- DVE custom masked_sq + TT(l*sign) + Act copy-accum.
      'dve'   -- all three passes on DVE (baseline structure).
    ('pool' is dead: walrus rejects DVE-class instructions on Pool.)"""
    key = ("nc2", repeats, loop, j3, dtype, bufs_in, bufs_work, bufs_ps, q2,
           tiles)
    if key in _NC_CACHE:
        return _NC_CACHE[key]
    ops = _get_custom_ops()

    f32 = mybir.dt.float32
    dt_in = mybir.dt.float16 if dtype == "f16" else mybir.dt.float32
    A = mybir.AluOpType

    nc = bacc.Bacc(
        "TRN2", target_bir_lowering=False, debug=False, num_devices=_NCORES
    )
    pk = nc.dram_tensor(
        "pk", [_SPC, _P, _NT, _F], dt_in, kind="ExternalInput"
    ).ap()
    # per-engine stats tiles (separate DRAM outs avoid cross-engine deps):
    #   sv[:, 2*s+mi] = S_tot          (Act square accum / DVE custom accum)
    #   sp[:, 2*s+mi] = S_pos-carrier  (DVE accum; unused in 'pe')
    #   sa[:, 2*s+mi] = n_pos-carrier  (DVE ts accum in 'pe': n_pos;
    #                                   Act sign accum else: 2*n_pos - N)
    stats_v = nc.dram_tensor("stats_v", [_P, 8], f32, kind="ExternalOutput").ap()
    stats_p = nc.dram_tensor("stats_p", [_P, 8], f32, kind="ExternalOutput").ap()
    stats_a = nc.dram_tensor("stats_a", [_P, 8], f32, kind="ExternalOutput").ap()
    if j3 == "pe":
        # W matrices, up to two [128,128] fp16 per sample-map (slots
        # 2*col, 2*col+1); host reads the diagonals
        stats_w = nc.dram_tensor(
            "stats_w", [_P, 16, 128], dt_in, kind="ExternalOutput").ap()

    with tile.TileContext(nc) as tc, ExitStack() as ctx:
        inpool = ctx.enter_context(tc.tile_pool(name="in", bufs=bufs_in))
        workpool = ctx.enter_context(tc.tile_pool(name="work", bufs=bufs_work))
        latepool = ctx.enter_context(tc.tile_pool(name="late", bufs=3))
        statpool = ctx.enter_context(tc.tile_pool(name="stat", bufs=1))
        pspool = None
        if j3 == "pe":
            pspool = ctx.enter_context(tc.psum_pool(name="ps", bufs=bufs_ps))

        sv = statpool.tile([_P, 8], f32)
        sp = statpool.tile([_P, 8], f32)
        sa = statpool.tile([_P, 8], f32)
        wout = None
        if j3 == "pe":
            wout = statpool.tile([_P, 16, 128], dt_in)
            nc.scalar.memzero(sv[:])
            nc.scalar.memzero(sa[:])
            nc.vector.memzero(sp[:])
        else:
            nc.scalar.memzero(sa[:])
            nc.vector.memzero(sp[:])
            nc.vector.memzero(sv[:])

        sign_bias = {}
        # eps = 2^-24 shifts the threshold off the fp16/f32 input grid
        # so sign() reproduces the strict '>' exactly (never hits 0).
        for thr in (_T_G, _T_A):
            bt = statpool.tile([_P, 1], f32, tag=f"bias{int(thr * 100)}")
            nc.gpsimd.memset(bt[:], -(_f32_exact(thr) + 2.0 ** -24))
            sign_bias[thr] = bt
        thr_full = {}
        if j3 == "pe":
            for thr in (_T_G, _T_A):
                tf = statpool.tile([_P, _F], dt_in, tag=f"thrF{int(thr * 100)}")
                nc.gpsimd.memset(tf[:], _f32_exact(thr))
                thr_full[thr] = tf

        # Engine balance (measured ns/op): DVE custom 2389, TT 1049;
        # Act ~2400-2630/op; PE trace job ~1261.  A-maps put the threshold
        # test on Act (Sign), C-maps on DVE (TT is_gt); x=3 C-maps makes
        # DVE ~= Act ~= 32us, under the ~34us DMA roof.
        # PSUM -> SBUF copies are deferred so the in-order DVE stream never
        # blocks on a PE accumulation chain: flush oldest only when >3
        # pending (PE is then several maps ahead), rest at body end.
        pending_w = []

        def flush_w(limit):
            while len(pending_w) > limit:
                ps, wc = pending_w.pop(0)
                nc.vector.tensor_copy(out=wout[:, wc], in_=ps[:])

        def pe_trace_job(lhs, rhs, wcol):
            ps = pspool.tile([_P, 128], f32, tag="w")
            for k in range(16):
                c = slice(k * 128, (k + 1) * 128)
                nc.tensor.matmul(ps[:], lhsT=lhs[:, c], rhs=rhs[:, c],
                                 start=(k == 0), stop=(k == 15))
            pending_w.append((ps, wcol))
            flush_w(3)

        def emit_sample_pe5(sub, s):
            maps = [(0, sub[0][:], sub[3][:], _f32_exact(_T_G), _T_G, s * 2),
                    (1, sub[1][:], sub[4][:], _f32_exact(_T_A), _T_A,
                     s * 2 + 1)]
            _emit_maps_pe(maps, sub[2][:], s)

        def emit_sample_pe(tA, tB, rm_ap, s):
            maps = []
            for mi, (gi, pi, thr) in enumerate(
                ((0, 0, _T_G), (1, 1, _T_A))
            ):
                col = s * 2 + mi
                maps.append((mi, tA[:, gi], tB[:, pi], _f32_exact(thr), thr,
                             col))
            _emit_maps_pe(maps, rm_ap, s)

        def _emit_maps_pe(maps, rm_t, s):
            # Act first: A-map sign tensors depend only on the DMA
            zs_t = {}
            for mi, gt_t, pr_t, thr32, thr, col in maps:
                if col not in _C_SET:
                    zs = workpool.tile([_P, _F], dt_in, tag=f"zs{mi}")
                    nc.scalar.activation(
                        zs[:], gt_t, mybir.ActivationFunctionType.Sign,
                        bias=sign_bias[thr][:], scale=1.0,
                        accum_out=sa[:, col : col + 1],
                    )
                    zs_t[col] = zs
            # DVE 2x first: C-map p = (gt > t) in {0,1} -- cheap, and
            # keeps the input tile's gt readers early so it releases fast
            p_t = {}
            for mi, gt_t, pr_t, thr32, thr, col in maps:
                if col in _C_SET:
                    p = workpool.tile([_P, _F], dt_in, tag=f"p{mi}")
                    nc.vector.tensor_tensor(
                        out=p[:], in0=gt_t, in1=thr_full[thr][:], op=A.is_gt)
                    p_t[col] = p
            # DVE: d = (pred - gt) - (gt > t) * relu(pred - 1); d' = d*sqrt(m)
            dp_t = {}
            for mi, gt_t, pr_t, thr32, thr, col in maps:
                d = workpool.tile([_P, _F], dt_in, tag=f"d{mi}")
                nc.vector._custom_dve(
                    ops["clamped_diff"], out=d[:], in0=pr_t,
                    in1=gt_t, s0=thr32, s1=1.0,
                )
                dp = workpool.tile([_P, _F], dt_in, tag=f"dp{mi}")
                nc.vector.tensor_tensor(out=dp[:], in0=d[:], in1=rm_t,
                                        op=A.mult)
                dp_t[col] = dp
            # Act: sq = d'^2 = d^2 * m, accum -> S_tot
            for mi, gt_t, pr_t, thr32, thr, col in maps:
                sq = latepool.tile([_P, _F], dt_in, tag=f"sq{mi}")
                nc.scalar.activation(
                    sq[:], dp_t[col][:],
                    mybir.ActivationFunctionType.Square,
                    bias=0.0, scale=1.0, accum_out=sv[:, col : col + 1],
                )
                # PE trace jobs
                if col in _C_SET:
                    pe_trace_job(p_t[col], sq, 2 * col)       # S_pos
                    pe_trace_job(p_t[col], p_t[col], 2 * col + 1)  # n_pos
                else:
                    pe_trace_job(zs_t[col], sq, 2 * col)      # 2*S_pos-S_tot

        def emit_map_v2(t, s, mi, gi, pi, thr):
            gt_t = t[:, gi]
            pr_t = t[:, pi]
            m_t = t[:, _I_M]
            thr32 = _f32_exact(thr)
            col = s * 2 + mi

            # Act: zs = sign(gt - t - eps), accum -> 2*n_pos - N
            zs = workpool.tile([_P, _F], dt_in, tag="zs")
            nc.scalar.activation(
                zs[:], gt_t, mybir.ActivationFunctionType.Sign,
                bias=sign_bias[thr][:], scale=1.0,
                accum_out=sa[:, col : col + 1],
            )
            # DVE: d = (pred - gt) - (gt > t) * relu(pred - 1)
            d = workpool.tile([_P, _F], dt_in, tag="d")
            nc.vector._custom_dve(
                ops["clamped_diff"], out=d[:], in0=pr_t,
                in1=gt_t, s0=thr32, s1=1.0,
            )
            # DVE: l = d^2 * m, accum -> S_tot
            l = workpool.tile([_P, _F], dt_in, tag="l")
            nc.vector._custom_dve(
                ops["masked_sq"], out=l[:], in0=d[:], in1=m_t,
                accum_out=sv[:, col : col + 1],
            )
            if j3 == "ttact":
                # u = l * zs (2x TT); Act copy-accum -> 2*S_pos - S_tot
                u = workpool.tile([_P, _F], dt_in, tag="u")
                nc.vector.tensor_tensor(
                    out=u[:], in0=l[:], in1=zs[:], op=A.mult,
                )
                dump = workpool.tile([_P, _F], dt_in, tag="dump")
                nc.scalar.activation(
                    dump[:], u[:], mybir.ActivationFunctionType.Copy,
                    bias=0.0, scale=1.0,
                    accum_out=sp[:, col : col + 1],
                )
            else:  # 'dve'
                z = workpool.tile([_P, _F], dt_in, tag="z")
                nc.vector.scalar_tensor_tensor(
                    out=z[:], in0=gt_t, scalar=thr32, in1=l[:],
                    op0=A.is_gt, op1=A.mult,
                    accum_out=sp[:, col : col + 1],
                )

        def emit_sample(s):
            if j3 == "pe":
                q2_eng = {"sync": nc.sync, "scalar": nc.scalar,
                          "gpsimd": nc.gpsimd}[q2]
                if tiles == "one":
                    tt = inpool.tile([_P, _NT, _F], dt_in, tag="pk1")
                    nc.sync.dma_start(out=tt[:], in_=pk[s])
                    tA, tB, rm_ap = tt[:, 0:3], tt[:, 3:5], tt[:, 2]
                elif tiles == "five":
                    sub = []
                    for j, tg in enumerate(("tgr", "tga", "trm", "tpr", "tpa")):
                        st = inpool.tile([_P, _F], dt_in, tag=tg)
                        eng = q2_eng if j >= 3 else nc.sync
                        eng.dma_start(out=st[:], in_=pk[s, :, j])
                        sub.append(st)
                    tA, tB, rm_ap = None, None, None
                    emit_sample_pe5(sub, s)
                    return
                else:  # "two"
                    tA_t = inpool.tile([_P, 3, _F], dt_in, tag="pkA")
                    nc.sync.dma_start(out=tA_t[:], in_=pk[s, :, 0:3])
                    tB_t = inpool.tile([_P, 2, _F], dt_in, tag="pkB")
                    q2_eng.dma_start(out=tB_t[:], in_=pk[s, :, 3:5])
                    tA, tB, rm_ap = tA_t[:], tB_t[:], tA_t[:, 2]
                emit_sample_pe(tA, tB, rm_ap, s)
                return
            t = inpool.tile([_P, _NT, _F], dt_in, tag="pk")
            nc.sync.dma_start(out=t[:], in_=pk[s])
            if False:
                pass
            else:
                for mi, (gi, pi, thr) in enumerate(
                    ((_I_RGT, _I_RPRED, _T_G), (_I_AGT, _I_APRED, _T_A))
                ):
                    emit_map_v2(t, s, mi, gi, pi, thr)

        def emit_body():
            for s in range(_SPC):
                emit_sample(s)
            if j3 == "pe":
                flush_w(0)

        if loop and repeats > 1:
            with tc.For_i(0, repeats, 1):
                emit_body()
        else:
            for _ in range(repeats):
                emit_body()

        nc.sync.dma_start(out=stats_v[:], in_=sv[:])
        nc.sync.dma_start(out=stats_p[:], in_=sp[:])
        nc.sync.dma_start(out=stats_a[:], in_=sa[:])
        if j3 == "pe":
            nc.sync.dma_start(out=stats_w[:], in_=wout[:])

    nc.compile()
    _NC_CACHE[key] = nc
    return nc


# --------------------------------------------------------- host-side packing


def _pack_inputs(arr32, j3="pe"):
    """arr32: dict of [B, P, F] float32 -> list of per-core packed
    [SPC, P, NT, F] arrays in the kernel input dtype.  For the 'pe'
    design the mask slot carries sqrt(mask) so that the device's
    Square(d * sqrt(m)) equals d^2 * m."""
    dt = np.float16
    mk = arr32["mask"]
    if j3 == "pe":
        mk = np.sqrt(mk)
    packed = []
    for c in range(_NCORES):
        sl = slice(c * _SPC, (c + 1) * _SPC)
        buf = np.empty((_SPC, _P, _NT, _F), dtype=dt)
        buf[:, :, _I_RGT] = arr32["region_score_gt"][sl]
        buf[:, :, _I_RPRED] = arr32["region_score_pred"][sl]
        buf[:, :, _I_AGT] = arr32["affinity_score_gt"][sl]
        buf[:, :, _I_APRED] = arr32["affinity_score_pred"][sl]
        buf[:, :, _I_M] = mk[sl]
        packed.append(buf)
    return packed


# ------------------------------------------------------------ host fallback


def _host_sample_loss(pre_loss, label, thresh):
    """Exact per-sample replica of reference._single_image_loss (one sample)."""
    pre_loss = pre_loss.astype(np.float64).ravel()
    label = label.astype(np.float32).ravel()
    pos_mask = label > np.float32(thresh)
    n_pos = int(pos_mask.sum())
    n_neg = pre_loss.size - n_pos
    if n_pos == 0:
        top = np.sort(pre_loss)[::-1][:_TOPK_FALLBACK]
        return float(top.mean())
    pos_loss = pre_loss[pos_mask].sum() / n_pos
    k = min(3 * n_pos, n_neg)
    if k <= 0:
        return float(pos_loss)
    neg_vals = np.sort(pre_loss[~pos_mask])[::-1]
    neg_loss = neg_vals[:k].sum() / k
    return float(pos_loss + neg_loss)


def _host_pre_loss(gt, pred, mask, thresh):
    gt = gt.astype(np.float32)
    pred = pred.astype(np.float32)
    clamped = np.where((gt > np.float32(thresh)) & (pred > np.float32(1.0)),
                       np.float32(1.0), pred)
    d = clamped.astype(np.float64) - gt.astype(np.float64)
    return d * d * mask.astype(np.float64)


# ------------------------------------------------------------------- bench


def _io_spec(nc):
    """Mirror run_bass_via_pjrt's input/output discovery."""
    partition_name = (
        nc.partition_id_tensor.name if nc.partition_id_tensor else None
    )
    in_names, out_names, out_avals, zero_outs = [], [], [], []
    import jax

    for alloc in nc.m.functions[0].allocations:
        if not isinstance(alloc, mybir.MemoryLocationSet):
            continue
        name = alloc.memorylocations[0].name
        if alloc.kind == "ExternalInput":
            if name != partition_name:
                in_names.append(name)
        elif alloc.kind == "ExternalOutput":
            out_names.append(name)
            shape = tuple(alloc.tensor_shape)
            dtype = mybir.dt.np(alloc.dtype)
            out_avals.append(jax.core.ShapedArray(shape, dtype))
            zero_outs.append(np.zeros(shape, dtype))
    return partition_name, in_names, out_names, out_avals, zero_outs


def _bench_one(inputs, iters=30, warmup=2, **build_kw):
    """Amortized per-execution wall time (ns) over `iters` queued runs."""
    import time
    import jax
    from jax.sharding import Mesh, PartitionSpec
    from jax.experimental.shard_map import shard_map
    from concourse import bass2jax
    from concourse.bass2jax import _bass_exec_p, install_neuronx_cc_hook

    install_neuronx_cc_hook()
    nc = _build_bass(**build_kw)
    pname, in_names, out_names, out_avals, zero_outs = _io_spec(nc)
    n_params, n_outs = len(in_names), len(out_names)
    all_names = in_names + out_names + ([pname] if pname else [])

    def _body(*args):
        operands = list(args)
        if pname is not None:
            operands.append(bass2jax.partition_id_tensor())
        outs = _bass_exec_p.bind(
            *operands,
            out_avals=tuple(out_avals),
            in_names=tuple(all_names),
            out_names=tuple(out_names),
            lowering_input_output_aliases=(),
            sim_require_finite=True,
            sim_require_nnan=True,
            nc=nc,
        )
        return tuple(outs)

    devices = jax.devices()[:_NCORES]
    mesh = Mesh(np.asarray(devices), ("core",))
    in_specs = (PartitionSpec("core"),) * (n_params + n_outs)
    out_specs = (PartitionSpec("core"),) * n_outs
    donate = tuple(range(n_params, n_params + n_outs))
    sharded = jax.jit(
        shard_map(_body, mesh=mesh, in_specs=in_specs, out_specs=out_specs,
                  check_rep=False),
        donate_argnums=donate, keep_unused=True,
    )

    arr32 = {k: np.ascontiguousarray(
        np.asarray(v, np.float32).reshape(_B, _P, _F))
        for k, v in inputs.items()}
    packed = _pack_inputs(arr32, j3=build_kw.get("j3", "pe"))
    assert in_names == ["pk"], in_names
    concat_in = [np.concatenate(packed, axis=0)]  # [NCORES*SPC, P, NT, F]
    dev_in = [jax.device_put(a) for a in concat_in]

    def zeros():
        return [np.zeros((_NCORES * z.shape[0], *z.shape[1:]), z.dtype)
                for z in zero_outs]

    for _ in range(warmup):
        outs = sharded(*dev_in, *zeros())
        jax.block_until_ready(outs)
    zs = [zeros() for _ in range(iters)]
    t0 = time.perf_counter()
    results = [sharded(*dev_in, *z) for z in zs]
    jax.block_until_ready(results)
    t1 = time.perf_counter()
    return (t1 - t0) / iters * 1e9


def bench(inputs, rounds=3, k_lo=400, k_hi=1200, **build_kw):
    """Device time per kernel body (ns): slope between on-device For_i loops
    of k_lo and k_hi iterations.  K must be large enough that device time
    dominates the dispatch roundtrip, else async dispatch hides it."""
    est = []
    build_kw.setdefault("j3", os.environ.get("MAPLOSS_J3", "pe"))
    for _ in range(rounds):
        lo = _bench_one(inputs, iters=4, repeats=k_lo, loop=True, **build_kw)
        hi = _bench_one(inputs, iters=4, repeats=k_hi, loop=True, **build_kw)
        est.append((hi - lo) / (k_hi - k_lo))
    return float(np.median(est))


# ------------------------------------------------------------------- kernel

LAST_RESULTS = None


def kernel(**inputs):
    global LAST_RESULTS
    arr32 = {
        k: np.ascontiguousarray(
            np.asarray(v, dtype=np.float32).reshape(_B, _P, _F))
        for k, v in inputs.items()
    }
    j3 = os.environ.get("MAPLOSS_J3", "pe")
    nc = _build_bass(j3=j3)

    packed = _pack_inputs(arr32, j3=j3)
    in_maps = [{"pk": packed[c]} for c in range(_NCORES)]

    res = bass_utils.run_bass_kernel_spmd(
        nc, in_maps, core_ids=list(range(_NCORES))
    )
    LAST_RESULTS = res

    # ---- host-side finish (tiny): per-sample scalars ----------------------
    per_sample = np.zeros((2, _B), dtype=np.float64)   # [map, sample]
    fallback_samples = []
    for c in range(_NCORES):
        sv = res.results[c]["stats_v"].astype(np.float64).sum(axis=0)  # [8]
        sp = res.results[c]["stats_p"].astype(np.float64).sum(axis=0)  # [8]
        sa = res.results[c]["stats_a"].astype(np.float64).sum(axis=0)  # [8]
        if j3 == "pe":
            w = res.results[c]["stats_w"].astype(np.float64)  # [128,16,128]
            tr = np.einsum("psp->s", w)                       # [16] diagonals
        for s in range(_SPC):
            b = c * _SPC + s
            for mi in range(2):
                col = s * 2 + mi
                S_tot = sv[col]
                if j3 == "pe":
                    if col in _C_SET:
                        S_pos = tr[2 * col]
                        n_pos_f = tr[2 * col + 1]
                    else:
                        S_pos = (tr[2 * col] + S_tot) / 2.0
                        n_pos_f = (sa[col] + _N) / 2.0
                elif j3 == "ttact":
                    # sp holds sum(l * sign) = 2*S_pos - S_tot
                    S_pos = (sp[col] + S_tot) / 2.0
                    n_pos_f = (sa[col] + _N) / 2.0
                else:
                    S_pos = sp[col]
                    n_pos_f = (sa[col] + _N) / 2.0
                n_pos = int(round(n_pos_f))
                n_neg = _N - n_pos
                ok = abs(n_pos_f - n_pos) < 1e-3
                if ok and n_pos > 0 and (n_neg == 0 or 3 * n_pos >= n_neg):
                    pos_loss = S_pos / n_pos
                    neg_loss = (S_tot - S_pos) / n_neg if n_neg > 0 else 0.0
                    per_sample[mi, b] = pos_loss + neg_loss
                else:
                    fallback_samples.append((mi, b))

    if fallback_samples:
        rgt = arr32["region_score_gt"]
        agt = arr32["affinity_score_gt"]
        rpred = arr32["region_score_pred"]
        apred = arr32["affinity_score_pred"]
        m = arr32["mask"]
        for mi, b in fallback_samples:
            if mi == 0:
                pl = _host_pre_loss(rgt[b], rpred[b], m[b], _T_G)
                per_sample[mi, b] = _host_sample_loss(pl, rgt[b], _T_G)
            else:
                pl = _host_pre_loss(agt[b], apred[b], m[b], _T_A)
                per_sample[mi, b] = _host_sample_loss(pl, agt[b], _T_A)

    char_loss = per_sample[0].sum()
    affi_loss = per_sample[1].sum()
    out = _LAMBDA * char_loss / _B + affi_loss / _B
    return np.float32(out)
